# revision 5
# baseline (speedup 1.0000x reference)
"""Trainium2 Bass kernel v2: fp8 QK projection + block-diag sim attention.

Problem shapes (hardcoded from the task spec):
  x:        (2, 1024, 64, 512) fp32
  w_qkv:    (1536, 512) fp32   -> q|k|v each 512 feats = 8 heads x 64
  pos_bias: (8, 64, 64) fp32
  focus_present_mask: (2,) bool

Key design points vs v1 (313 us):
  - QK projection runs in fp8e4 DoubleRow mode. Measured on this HW the DR
    matmuls stream at 1 row/cycle (same as bf16) -- no PE win -- but fp8
    halves the x DMA traffic and SBUF footprint for the QK stream. Weights
    are pre-scaled by 128 (power of 2) to dodge fp8 subnormals; the 2^-17
    compensation folds into the exp activation's scale for free. End-to-end
    rel_fro = 3.6e-3 (bf16 baseline 2.7e-3), verified on HW.
  - sim matmuls use block-diagonal k (kbd) packing: one [128,128]x[128,64]
    matmul covers two positions (PE cost is rows-only => half the rows).
  - QK PSUM: [64, 8,2,64] tiles (DoubleRow out is fixed at partitions 0:63),
    ring of 2 (4 banks). Each 512-token group is one PSUM bank written by ONE
    matmul per kp (1024-free moving): start=True marks its whole 2KB bank
    pending-zero, so accumulation groups must own a full bank.
  - V projections and PV stay bf16 (V feeds the output directly; fp8 there
    measured 3.7e-2 > 2e-2 tolerance).
  - PV for pair g is emitted one pair-slot late, with projection matmuls in
    between, so the in-order PE queue never waits on the exp->bias chain.
  - PE p-state warm-up: dummy matmul train under the initial DMA.
  - GpSimd does the (all-SBUF) bias multiply; PSUM drains go to Scalar and
    Vector only (GPSIMD cannot access PSUM).
"""

import numpy as np

import concourse.bass as bass
import concourse.bacc as bacc
import concourse.mybir as mybir
import concourse.tile as tile
from concourse.bass_utils import run_bass_kernel_spmd

HEADS = 8
DH = 64
NTOK = 64          # tokens per spatial position
DIM = 512
N_CORES = 8
P = 128
BLK = 1024         # tokens per block = 16 positions = 8 pairs
F32 = mybir.dt.float32
BF16 = mybir.dt.bfloat16
F8 = mybir.dt.float8e4

W_SCALE = 128.0            # fp8 weight pre-scale (power of 2)
EXP_SCALE = 1.0 / (W_SCALE * W_SCALE * 8.0)   # undo w scales + softmax 1/sqrt(64)

LAST_RESULT = None
_KERNEL_CACHE: dict = {}


def _ensure_ntff_hook():
    """Make BASS_TRACE=1 usable: bass_utils' axon trace path imports
    antenv.axon_hooks, which some images lack."""
    import sys
    import types

    try:
        import antenv.axon_hooks  # noqa: F401

        return
    except ImportError:
        pass
    try:
        import antenv
        from trn_agent_boot.trn_boot import _ntff_profile_via_ctypes
    except ImportError:
        return
    mod = types.ModuleType("antenv.axon_hooks")
    _state = {"hook": None}
    mod.set_axon_ntff_profile_hook = lambda h: _state.__setitem__("hook", h)
    mod.get_axon_ntff_profile_hook = lambda: _state["hook"]
    sys.modules["antenv.axon_hooks"] = mod
    antenv.axon_hooks = mod
    import os as _os

    so = "/opt/axon/libaxon_pjrt.so"
    if _os.path.exists(so):
        try:
            mod.set_axon_ntff_profile_hook(_ntff_profile_via_ctypes(so))
        except Exception:
            pass


class _Sched:
    """Emission helper holding engine rotation state."""

    def __init__(self, nc):
        self.nc = nc
        self.drain_rr = 0

    def drain_engine(self):
        # PSUM drains: only Scalar and Vector may read PSUM (GpSimd cannot)
        engines = ("scalar", "vector")
        e = engines[self.drain_rr % len(engines)]
        self.drain_rr += 1
        return e

    def copy(self, out, in_, engine):
        nc = self.nc
        if engine == "scalar":
            nc.scalar.copy(out=out, in_=in_)
        elif engine == "vector":
            nc.vector.tensor_copy(out=out, in_=in_)
        else:
            nc.gpsimd.tensor_copy(out=out, in_=in_)


def _build_kernel(a_tok: int, v_tok: int):
    """a_tok, v_tok: tokens per core needing attention / V-only (mult of BLK)."""
    nc = bacc.Bacc("TRN2")
    EXP = mybir.ActivationFunctionType.Exp
    DR = mybir.MatmulPerfMode.DoubleRow

    nb_a = a_tok // BLK
    nb_v = v_tok // BLK

    wqk8 = nc.dram_tensor("wqk8", [DIM, 2 * DIM], F8, kind="ExternalInput")
    wvT = nc.dram_tensor("wvT", [DIM, DIM], BF16, kind="ExternalInput")
    ebiasT = nc.dram_tensor("ebiasT", [P, DIM], BF16, kind="ExternalInput")
    xa8T = xaT = out_a = None
    if a_tok:
        xa8T = nc.dram_tensor("xa8T", [DIM, a_tok], F8, kind="ExternalInput")
        xaT = nc.dram_tensor("xaT", [DIM, a_tok], BF16, kind="ExternalInput")
        out_a = nc.dram_tensor("out_a", [a_tok, DIM], BF16, kind="ExternalOutput")
    xvT = out_v = None
    if v_tok:
        xvT = nc.dram_tensor("xvT", [DIM, v_tok], BF16, kind="ExternalInput")
        out_v = nc.dram_tensor("out_v", [v_tok, DIM], BF16, kind="ExternalOutput")

    with tile.TileContext(nc) as tc:
        with (
            tc.tile_pool(name="const", bufs=1) as const,
            tc.tile_pool(name="x8", bufs=2) as x8pool,
            tc.tile_pool(name="x16", bufs=2) as x16pool,
            tc.tile_pool(name="xv", bufs=2) as xvpool,
            tc.tile_pool(name="qst", bufs=2) as qstpool,
            tc.tile_pool(name="kbd", bufs=1) as kbdpool,
            tc.tile_pool(name="vt", bufs=1) as vtpool,
            tc.tile_pool(name="ee", bufs=2) as epool,
            tc.tile_pool(name="ot", bufs=2) as otpool,
            tc.tile_pool(name="ov", bufs=2) as ovpool,
            tc.tile_pool(name="rr", bufs=4) as rpool,
            tc.tile_pool(name="pq", bufs=2, space="PSUM") as pp_qk,
            tc.tile_pool(name="pv", bufs=1, space="PSUM") as pp_proj,
            tc.tile_pool(name="psx", bufs=1, space="PSUM") as pp_s,
            tc.tile_pool(name="pox", bufs=1, space="PSUM") as pp_o,
        ):
            sched = _Sched(nc)

            # ---- persistent SBUF tiles
            warm_sb = const.tile([P, DH], BF16)
            wv_sb = const.tile([P, 4, DIM], BF16)
            wqk8_sb = const.tile([P, 2, 2, 2 * DIM], F8)
            ebias_sb = const.tile([P, DIM], BF16)

            # ---- PE warm-up first: memset on vector BEFORE the big kbd
            # memsets so the dummy matmul train starts immediately.
            nc.vector.memset(warm_sb[:], 0.0)
            pwarm = pp_s.tile([DH, DH], F32, tag="ps_s", name="pwarm")
            for _ in range(96):
                nc.tensor.matmul(
                    pwarm[:],
                    lhsT=warm_sb[:, 0:64],
                    rhs=warm_sb[:],
                    start=True,
                    stop=True,
                )

            # kbd: block-diag k, 2 ring slots; off-diag quadrants must be 0.
            kbd = [
                kbdpool.tile([P, HEADS, 8, P], BF16, tag=f"kbd{s}", name=f"kbd{s}")
                for s in range(2)
            ]
            nc.vector.memset(kbd[0][0:64, :, :, 64:128], 0.0)
            nc.gpsimd.memset(kbd[0][64:128, :, :, 0:64], 0.0)
            nc.scalar.memzero(kbd[1][0:64, :, :, 64:128])
            nc.gpsimd.memset(kbd[1][64:128, :, :, 0:64], 0.0)

            # vt: 16 persistent slots (2 blocks x 8 pairs); ones column written
            # once and reused (drains only touch cols 0:64)
            vts_all = [
                vtpool.tile([P, HEADS, 65], BF16, tag=f"vt{i}", name=f"vt{i}")
                for i in range(16)
            ]
            for i, vt in enumerate(vts_all):
                if i % 2 == 0:
                    nc.gpsimd.memset(vt[:, :, 64:65], 1.0)
                else:
                    nc.vector.memset(vt[:, :, 64:65], 1.0)

            # ---- input DMAs for block 0 + constants
            xa8_r = xa8T[:, :].rearrange("(kp i p) t -> p kp i t", kp=2, i=2) if a_tok else None
            xa16_r = xaT[:, :].rearrange("(k p) t -> p k t", p=P) if a_tok else None
            xv_r = xvT[:, :].rearrange("(k p) t -> p k t", p=P) if v_tok else None

            x8_t = x16_t = None
            if nb_a:
                x8_t = x8pool.tile([P, 2, 2, BLK], F8, tag="x8")
                nc.sync.dma_start(x8_t[:], xa8_r[:, :, :, 0:BLK])
            nc.sync.dma_start(
                wqk8_sb[:], wqk8[:, :].rearrange("(kp i p) e -> p kp i e", kp=2, i=2)
            )
            if nb_a:
                x16_t = x16pool.tile([P, 4, BLK], BF16, tag="x16")
                nc.sync.dma_start(x16_t[:], xa16_r[:, :, 0:BLK])
            wvT_r = wvT[:, :].rearrange("(k p) e -> p k e", p=P)
            for kt in range(4):
                nc.sync.dma_start(wv_sb[:, kt], wvT_r[:, kt])
            nc.sync.dma_start(ebias_sb[:], ebiasT[:, :])
            xv_t = None
            if nb_v:
                xv_t = xvpool.tile([P, 4, BLK], BF16, tag="xv")
                nc.sync.dma_start(xv_t[:], xv_r[:, :, 0:BLK])

            def qk_fb(fb, x8_tile, qst_t, kbd_t):
                """One feature-block (64 feats) of the fp8 DR QK projection.

                Each 512-token group is exactly one PSUM bank and is written by
                ONE matmul per kp (1024-free moving): a start=True write marks
                its whole 2KB bank pending-zero, so accumulation groups must
                own a full bank. kp-outer keeps LDWEIGHTS at 2 per fb."""
                rows = pp_qk.tile([DH, 8, 2, DH], F32, tag="pq", name="pq")
                for kp in range(2):
                    for g2 in range(2):
                        nc.tensor.matmul(
                            rows[:, g2 * 4 : (g2 + 1) * 4],
                            lhsT=wqk8_sb[:, kp, :, fb * 64 : (fb + 1) * 64],
                            rhs=x8_tile[:, kp, :, g2 * 512 : (g2 + 1) * 512],
                            start=(kp == 0),
                            stop=(kp == 1),
                            perf_mode=DR,
                            skip_group_check=True,
                        )
                # drains: A-positions -> partitions 0:64, B -> 64:128
                if fb < 8:
                    h = fb
                    sched.copy(qst_t[0:64, h], rows[:, :, 0, :], sched.drain_engine())
                    sched.copy(qst_t[64:128, h], rows[:, :, 1, :], sched.drain_engine())
                else:
                    h = fb - 8
                    sched.copy(
                        kbd_t[0:64, h, :, 0:64], rows[:, :, 0, :], sched.drain_engine()
                    )
                    sched.copy(
                        kbd_t[64:128, h, :, 64:128],
                        rows[:, :, 1, :],
                        sched.drain_engine(),
                    )

            def vattn_tt(b, tt, x16_tile):
                """V projection for one token-pair (128 tokens) of an attn block."""
                psv = pp_proj.tile([P, DIM], F32, tag="psv", name="psv")
                for kt in range(4):
                    nc.tensor.matmul(
                        psv[:],
                        lhsT=x16_tile[:, kt, tt * 128 : (tt + 1) * 128],
                        rhs=wv_sb[:, kt, :],
                        start=(kt == 0),
                        stop=(kt == 3),
                    )
                vt = vts_all[(b % 2) * 8 + tt]
                eng = "vector" if tt % 2 == 0 else "scalar"
                sched.copy(
                    vt[:, :, 0:64], psv[:].rearrange("p (h d) -> p h d", h=HEADS), eng
                )

            def vonly_tt(vb, tt, xv_tile, ov_t):
                psv = pp_proj.tile([P, DIM], F32, tag="psv", name="psv")
                for kt in range(4):
                    nc.tensor.matmul(
                        psv[:],
                        lhsT=xv_tile[:, kt, tt * 128 : (tt + 1) * 128],
                        rhs=wv_sb[:, kt, :],
                        start=(kt == 0),
                        stop=(kt == 3),
                    )
                eng = "scalar" if tt % 2 == 0 else "vector"
                sched.copy(ov_t[:, tt, :], psv[:], eng)
                nc.sync.dma_start(
                    out_v[vb * BLK + tt * 128 : vb * BLK + (tt + 1) * 128, :],
                    ov_t[:, tt, :],
                )

            def sim_part(g, qst_t, kbd_t):
                """sim matmuls + exp + bias for pair g; returns e_t."""
                pss = pp_s.tile([P, DIM], F32, tag="ps_s", name="pss")
                for h in range(HEADS):
                    nc.tensor.matmul(
                        pss[:, h * 64 : (h + 1) * 64],
                        lhsT=kbd_t[:, h, g, :],
                        rhs=qst_t[:, h, g, :],
                        start=True,
                        stop=True,
                    )
                e_raw = epool.tile([P, DIM], BF16, tag="Eraw", name="e_raw")
                nc.scalar.activation(e_raw[:], pss[:], EXP, scale=EXP_SCALE)
                e_t = epool.tile([P, DIM], BF16, tag="E", name="e_t")
                # all-SBUF elementwise -> GpSimd (off the PE critical path now)
                nc.gpsimd.tensor_tensor(
                    e_t[:], e_raw[:], ebias_sb[:], mybir.AluOpType.mult
                )
                return e_t

            def store_attn_block(b, ot_t):
                row0 = b * BLK
                for hb in range(2):
                    src = ot_t[hb * 64 : (hb + 1) * 64, :, :].rearrange(
                        "t g (ab cc) -> t g ab cc", ab=2
                    )
                    dst = out_a[
                        row0 : row0 + BLK, hb * 256 : (hb + 1) * 256
                    ].rearrange("(g ab t) cc -> t g ab cc", g=8, ab=2)
                    nc.sync.dma_start(dst, src)

            def pv_part(ent):
                """PV + normalize for a queued pair (one slot behind sim)."""
                b, g, e_t, vt, ot_t = ent
                pvt = pp_o.tile([P, 2, 512], F32, tag="pvt", name="pvt")
                for h in range(HEADS):
                    hb, hh = h // 4, h % 4
                    for ab in range(2):
                        nc.tensor.matmul(
                            pvt[hb * 64 : (hb + 1) * 64, ab, hh * 65 : hh * 65 + 65],
                            lhsT=e_t[ab * 64 : (ab + 1) * 64, h * 64 : (h + 1) * 64],
                            rhs=vt[ab * 64 : (ab + 1) * 64, h, :],
                            start=True,
                            stop=True,
                            tile_position=(ab * 64, hb * 64),
                        )
                pvt_r = pvt[:, :, 0:260].rearrange("p ab (h x) -> p ab h x", h=4)
                rec = rpool.tile([P, 2, 4, 1], F32, tag="rec", name="rec")
                nc.vector.reciprocal(rec[:], pvt_r[:, :, :, 64:65])
                nc.vector.tensor_tensor(
                    ot_t[:, g].rearrange("p (ab h d) -> p ab h d", ab=2, h=4),
                    pvt_r[:, :, :, 0:64],
                    rec[:].to_broadcast((P, 2, 4, 64)),
                    mybir.AluOpType.mult,
                )
                if g == 7:
                    store_attn_block(b, ot_t)

            # ================= emission =================
            vonly_units = [(vb, tt) for vb in range(nb_v) for tt in range(8)]
            vidx = 0
            ov_t = None
            cur_xv = xv_t
            self_next = [None]

            def emit_vonly():
                nonlocal vidx, ov_t, cur_xv
                if vidx >= len(vonly_units):
                    return False
                vb, tt = vonly_units[vidx]
                if tt == 0:
                    ov_t = ovpool.tile([P, 8, DIM], BF16, tag="ov", name="ov")
                vonly_tt(vb, tt, cur_xv, ov_t)
                vidx += 1
                # prefetch next vonly block's x a few units early; the tile
                # handle swap only happens at the block edge (tt == 7)
                if tt == 4 and vb + 1 < nb_v:
                    nxt = xvpool.tile([P, 4, BLK], BF16, tag="xv", name="xv")
                    nc.sync.dma_start(
                        nxt[:], xv_r[:, :, (vb + 1) * BLK : (vb + 2) * BLK]
                    )
                    self_next[0] = nxt
                if tt == 7 and vb + 1 < nb_v:
                    cur_xv = self_next[0]
                return True

            pv_queue = []

            def emit_pv():
                if pv_queue:
                    pv_part(pv_queue.pop(0))

            # --- prologue: QK(0) + Vattn(0), no attention yet
            qst = [None, None]
            if nb_a:
                qst[0] = qstpool.tile([P, HEADS, 8, DH], BF16, tag="qst", name="qst")
                for fb in range(16):
                    qk_fb(fb, x8_t, qst[0], kbd[0])
                    if fb % 2 == 1:
                        vattn_tt(0, fb // 2, x16_t)
                if nb_a > 1:
                    x8_t = x8pool.tile([P, 2, 2, BLK], F8, tag="x8", name="x8")
                    nc.sync.dma_start(x8_t[:], xa8_r[:, :, :, BLK : 2 * BLK])
                    x16_t = x16pool.tile([P, 4, BLK], BF16, tag="x16", name="x16")
                    nc.sync.dma_start(x16_t[:], xa16_r[:, :, BLK : 2 * BLK])

                for b in range(nb_a):
                    s = b % 2
                    ns = (b + 1) % 2
                    ot_t = otpool.tile([P, 8, DIM], BF16, tag="ot", name="ot")
                    if b + 1 < nb_a:
                        qst[ns] = qstpool.tile(
                            [P, HEADS, 8, DH], BF16, tag="qst", name="qst"
                        )
                    for g in range(8):
                        e_t = sim_part(g, qst[s], kbd[s])
                        pv_queue.append((b, g, e_t, vts_all[s * 8 + g], ot_t))
                        # filler between sim(g) and PV(g-1) hides the
                        # exp->bias chain from the in-order PE queue
                        if b + 1 < nb_a:
                            qk_fb(2 * g, x8_t, qst[ns], kbd[ns])
                            # V-only budget: 1/pair for blocks 0..nb_a-3,
                            # none in block nb_a-2 (it has full QK filler),
                            # 2/pair in the final block (which has none)
                            if b < nb_a - 2:
                                emit_vonly()
                            if len(pv_queue) > 1:
                                emit_pv()
                            qk_fb(2 * g + 1, x8_t, qst[ns], kbd[ns])
                            vattn_tt(b + 1, g, x16_t)
                        else:
                            emit_vonly()
                            if len(pv_queue) > 1:
                                emit_pv()
                            emit_vonly()
                    # prefetch block b+2 inputs
                    if b + 2 < nb_a:
                        x8_t = x8pool.tile([P, 2, 2, BLK], F8, tag="x8", name="x8")
                        nc.sync.dma_start(
                            x8_t[:], xa8_r[:, :, :, (b + 2) * BLK : (b + 3) * BLK]
                        )
                        x16_t = x16pool.tile([P, 4, BLK], BF16, tag="x16", name="x16")
                        nc.sync.dma_start(
                            x16_t[:], xa16_r[:, :, (b + 2) * BLK : (b + 3) * BLK]
                        )
            # epilogue: flush pending PV, then remaining V-only work
            while pv_queue:
                emit_pv()
                emit_vonly()
            while emit_vonly():
                pass

    nc.finalize()
    return nc


def _pad_positions(idx: np.ndarray, mult: int) -> np.ndarray:
    if len(idx) % mult == 0:
        return idx
    pad = mult - len(idx) % mult
    return np.concatenate([idx, np.full(pad, idx[-1], dtype=idx.dtype)])


def host_consts(w_qkv, pos_bias):
    import ml_dtypes

    bf16 = ml_dtypes.bfloat16
    f8 = ml_dtypes.float8_e4m3
    wq = w_qkv[0:512]
    wk = w_qkv[512:1024]
    wv = w_qkv[1024:1536]
    wqk8 = np.ascontiguousarray(
        np.clip(np.concatenate([wq, wk], axis=0).T * W_SCALE, -240, 240).astype(f8)
    )
    wvT = np.ascontiguousarray(wv.T.astype(bf16))
    # ebias[ab*64 + j, h*64 + i] = exp(pos_bias[h, i, j])
    big = np.zeros((64, 512), np.float32)
    for h in range(HEADS):
        big[:, h * 64 : (h + 1) * 64] = pos_bias[h].T
    ebiasT = np.ascontiguousarray(np.exp(np.tile(big, (2, 1))).astype(bf16))
    return wqk8, wvT, ebiasT


def kernel(x, w_qkv, pos_bias, focus_present_mask):
    global LAST_RESULT
    _ensure_ntff_hook()
    import ml_dtypes

    bf16 = ml_dtypes.bfloat16
    f8 = ml_dtypes.float8_e4m3

    x = np.ascontiguousarray(np.asarray(x), dtype=np.float32)
    w_qkv = np.asarray(w_qkv, dtype=np.float32)
    pos_bias = np.asarray(pos_bias, dtype=np.float32)
    mask = np.asarray(focus_present_mask).astype(bool)

    b, hw, n, dim = x.shape
    assert (n, dim) == (NTOK, DIM) and w_qkv.shape == (3 * HEADS * DH, DIM)
    x_flat = x.reshape(b * hw, n, dim)

    flat_idx = np.arange(b * hw)
    batch_of = flat_idx // hw
    attn_idx = flat_idx[~mask[batch_of]]
    vpr_idx = flat_idx[mask[batch_of]]

    # per-core granularity: 16 positions (one 1024-token block) x 8 cores
    attn_idx = _pad_positions(attn_idx, 16 * N_CORES) if len(attn_idx) else attn_idx
    vpr_idx = _pad_positions(vpr_idx, 16 * N_CORES) if len(vpr_idx) else vpr_idx
    a_pos_pc = len(attn_idx) // N_CORES
    v_pos_pc = len(vpr_idx) // N_CORES
    a_tok = a_pos_pc * NTOK
    v_tok = v_pos_pc * NTOK

    key = (a_tok, v_tok)
    if key not in _KERNEL_CACHE:
        _KERNEL_CACHE[key] = _build_kernel(a_tok, v_tok)
    nc = _KERNEL_CACHE[key]

    wqk8, wvT, ebiasT = host_consts(w_qkv, pos_bias)

    in_maps = []
    for core in range(N_CORES):
        m = {"wqk8": wqk8, "wvT": wvT, "ebiasT": ebiasT}
        if a_tok:
            ai = attn_idx[core * a_pos_pc : (core + 1) * a_pos_pc]
            xa = x_flat[ai].reshape(-1, DIM).T
            m["xa8T"] = np.ascontiguousarray(np.clip(xa, -240, 240).astype(f8))
            m["xaT"] = np.ascontiguousarray(xa.astype(bf16))
        if v_tok:
            vi = vpr_idx[core * v_pos_pc : (core + 1) * v_pos_pc]
            m["xvT"] = np.ascontiguousarray(x_flat[vi].reshape(-1, DIM).T.astype(bf16))
        in_maps.append(m)

    res = run_bass_kernel_spmd(nc, in_maps, core_ids=list(range(N_CORES)))
    LAST_RESULT = res

    out_flat = np.empty((b * hw, n, HEADS * DH), dtype=np.float32)
    for core in range(N_CORES):
        if a_tok:
            ai = attn_idx[core * a_pos_pc : (core + 1) * a_pos_pc]
            out_flat[ai] = (
                res.results[core]["out_a"]
                .astype(np.float32)
                .reshape(a_pos_pc, n, HEADS * DH)
            )
        if v_tok:
            vi = vpr_idx[core * v_pos_pc : (core + 1) * v_pos_pc]
            out_flat[vi] = (
                res.results[core]["out_v"]
                .astype(np.float32)
                .reshape(v_pos_pc, n, HEADS * DH)
            )
    return out_flat.reshape(b, hw, n, HEADS * DH)


# revision 6
# speedup vs baseline: 1.0435x; 1.0435x over previous
"""Trainium2 Bass kernel for fused QKV projection + per-head spatial attention.

Problem shapes (hardcoded from the task spec):
  x:        (2, 1024, 64, 512) fp32
  w_qkv:    (1536, 512) fp32   -> q|k|v each 512 feats = 8 heads x 64
  pos_bias: (8, 64, 64) fp32
  focus_present_mask: (2,) bool

Algorithm notes:
  - For a batch with focus_present_mask=True the mask is the identity ->
    softmax(sim masked to diag) == I exactly -> out = V = x @ w_v.T.
    Those positions only need the V projection.
  - For unfocused batches: full attention with pos_bias, no masking
    (mask is all-ones). Values are O(1) so exp() without amax shift is
    safe in fp32.
  - Sharding: data-parallel over (b*hw) positions across 8 cores.

Device kernel layout choices (no on-device transposes):
  - x is transposed on host -> xT (512, tokens): contraction dim on
    partitions for every matmul.
  - q^T,k^T produced feature-major (128 part = 2 heads x 64 dim), which
    directly feeds sim^T = (k^T).T @ q^T   (out: j on partitions, i free).
  - V produced token-major (128 part = 2 positions x 64 tokens) with a
    ones column appended, so PV = E.T @ [V|1] yields the softmax
    denominator as a per-partition column -> native tensor_tensor
    broadcast normalize.
  - sim^T for 16 (position, head) pairs packed into one PSUM bank pair;
    the pos-bias add is one tensor_tensor (GpSimd) against a
    host-precomputed (128, 512) constant; exp is one ScalarE activation
    per 16 pairs.
  - PV uses all 4 PE quadrants: out rows keyed by head-half (hb), PSUM
    bank keyed by position parity (p2); the output DMA un-permutes.
  - Attention chunks and V-only chunks are interleaved 1:1 so the
    DMA-heavy V-projection work rides under the compute-heavy attention
    work (keeps PE warm, hides stores).
  - All outputs stored bf16 (halves store traffic), cast to fp32 on host.
"""

import numpy as np

import concourse.bass as bass
import concourse.bacc as bacc
import concourse.mybir as mybir
import concourse.tile as tile
from concourse.bass_utils import run_bass_kernel_spmd

HEADS = 8
DH = 64
NTOK = 64          # tokens per spatial position
DIM = 512
QK_FEATS = 1024    # q + k feature columns
N_CORES = 8
P = 128
F32 = mybir.dt.float32
BF16 = mybir.dt.bfloat16

# test.py introspection: last BassKernelResults (exec_time_ns when BASS_TRACE=1)
LAST_RESULT = None

_KERNEL_CACHE: dict = {}


def _ensure_ntff_hook():
    """Make BASS_TRACE=1 usable: bass_utils' axon trace path imports
    antenv.axon_hooks, which some images lack. Provide the tiny get/set
    pair and register the ctypes NTFF hook the boot code would have."""
    import sys
    import types

    try:
        import antenv.axon_hooks  # noqa: F401

        return
    except ImportError:
        pass
    try:
        import antenv
        from trn_agent_boot.trn_boot import _ntff_profile_via_ctypes
    except ImportError:
        return
    mod = types.ModuleType("antenv.axon_hooks")
    _state = {"hook": None}
    mod.set_axon_ntff_profile_hook = lambda h: _state.__setitem__("hook", h)
    mod.get_axon_ntff_profile_hook = lambda: _state["hook"]
    sys.modules["antenv.axon_hooks"] = mod
    antenv.axon_hooks = mod
    import os as _os

    so = "/opt/axon/libaxon_pjrt.so"
    if _os.path.exists(so):
        try:
            mod.set_axon_ntff_profile_hook(_ntff_profile_via_ctypes(so))
        except Exception:
            pass


def _ecol(h):
    # exp-space column of head h: bank by head parity, then by h//2
    return (h % 2) * 256 + (h // 2) * 64


def _build_kernel(a_tok: int, v_tok: int):
    """Build the per-core Bass program.

    a_tok: tokens needing full attention on this core (multiple of 512, may be 0)
    v_tok: tokens needing only the V projection (multiple of 512, may be 0)
    """
    nc = bacc.Bacc("TRN2")

    wqkT = nc.dram_tensor("wqkT", [DIM, QK_FEATS], BF16, kind="ExternalInput")
    wvT = nc.dram_tensor("wvT", [DIM, DIM], BF16, kind="ExternalInput")
    ebiasT = nc.dram_tensor("ebiasT", [P, 512], BF16, kind="ExternalInput")
    xaT = out_a = None
    if a_tok:
        xaT = nc.dram_tensor("xaT", [DIM, a_tok], BF16, kind="ExternalInput")
        out_a = nc.dram_tensor("out_a", [a_tok, DIM], BF16, kind="ExternalOutput")
    xvT = out_v = None
    if v_tok:
        xvT = nc.dram_tensor("xvT", [DIM, v_tok], BF16, kind="ExternalInput")
        out_v = nc.dram_tensor("out_v", [v_tok, DIM], BF16, kind="ExternalOutput")

    EXP = mybir.ActivationFunctionType.Exp

    with tile.TileContext(nc) as tc:
        with tc.tile_pool(name="const", bufs=1) as const:
            # PE p-state warm-up: a dummy matmul train issued under the
            # initial input DMA so the real first matmuls run at 2.4 GHz
            # instead of ramping from 0.65 GHz (saves ~5us of ramp).
            warm_sb = const.tile([P, DH], BF16)
            nc.vector.memset(warm_sb[:], 0.0)
            # wv first: the V-projection is the first consumer
            wv_sb = const.tile([P, 4, DIM], BF16)
            wvT_r = wvT[:, :].rearrange("(k p) e -> p k e", p=P)
            # split by contraction tile so the very first matmul can start
            # as soon as its slice lands
            for kt in range(4):
                nc.sync.dma_start(wv_sb[:, kt], wvT_r[:, kt])
            # wqk/ebias tiles created here, but their loads are issued after
            # chunk 0's input DMA (they are needed a few us later)
            wqk_sb = const.tile([P, 4, QK_FEATS], BF16)
            ebias_sb = const.tile([P, 512], BF16)

            na = a_tok // 512
            nv = v_tok // 512
            with (
                tc.tile_pool(name="ax", bufs=4) as xpool,
                tc.tile_pool(name="aqk", bufs=2) as qkpool,
                tc.tile_pool(name="av", bufs=1) as vpool,
                tc.tile_pool(name="ae", bufs=4) as epool,
                tc.tile_pool(name="ao", bufs=2) as opool,
                tc.tile_pool(name="aov", bufs=2) as ovpool,
                tc.tile_pool(name="ar", bufs=8) as rpool,
                tc.tile_pool(name="app", bufs=2, space="PSUM") as pp_proj,
                tc.tile_pool(name="aps", bufs=1, space="PSUM") as pp_s,
                tc.tile_pool(name="apo", bufs=2, space="PSUM") as pp_o,
            ):
                # warm-up train (shares the pss PSUM slot via same pool+tag;
                # done long before the first sim matmul needs it)
                pwarm = pp_s.tile([DH, DH], F32, tag="ps_s", name="pwarm")
                for _ in range(96):
                    nc.tensor.matmul(
                        pwarm[:],
                        lhsT=warm_sb[:, 0:64],
                        rhs=warm_sb[:],
                        start=True,
                        stop=True,
                    )
                # software pipeline: chunk c+1's projections are emitted
                # interleaved with chunk c's attention groups, so each
                # engine queue alternates between PSUM-drain copies (which
                # unblock the PE) and the group chain ops (exp/e_mult) that
                # gate the PV matmuls.
                staged = None
                if na:
                    # chunk 0 input: per-kt slices, issued before wqk so the
                    # critical first bytes share DMA bandwidth with fewer
                    # competitors
                    xT0 = xpool.tile([P, 4, 512], BF16, tag="xT")
                    xaT_r0 = xaT[:, :].rearrange("(k p) t -> p k t", p=P)
                    for kt in range(4):
                        nc.sync.dma_start(xT0[:, kt], xaT_r0[:, kt, 0:512])
                nc.sync.dma_start(
                    wqk_sb[:], wqkT[:, :].rearrange("(k p) e -> p k e", p=P)
                )
                nc.sync.dma_start(ebias_sb[:], ebiasT[:, :])
                if na:
                    vts0, _ = _proj_v(
                        nc, 0, xaT, wv_sb, xpool, vpool, pp_proj, xT_pre=xT0
                    )
                    qkT0 = _proj_qk(nc, wqk_sb, qkpool, pp_proj, xT0, 0, 8)
                    staged = (vts0, qkT0)
                for c in range(max(na, nv)):
                    if c >= na:
                        _v_chunk(
                            nc, c, xvT, out_v, wv_sb, xpool, ovpool, pp_proj
                        )
                        continue
                    vts, qkT = staged
                    nxt_v = nxt_x = qkT_n = None
                    if c + 1 < na:
                        nxt_v, nxt_x = _proj_v(
                            nc, c + 1, xaT, wv_sb, xpool, vpool, pp_proj
                        )
                    ot = opool.tile([P, 2048], BF16, tag="ot")
                    # qk-projection split into 2-ft parts between groups:
                    # each group's exp then queues behind at most two
                    # PSUM-drain copies on ScalarE instead of four.
                    _group(nc, 0, ebias_sb, vts, qkT, ot,
                           epool, rpool, pp_s, pp_o, EXP)
                    if c + 1 < na:
                        qkT_n = _proj_qk(
                            nc, wqk_sb, qkpool, pp_proj, nxt_x, 0, 2
                        )
                    _group(nc, 1, ebias_sb, vts, qkT, ot,
                           epool, rpool, pp_s, pp_o, EXP)
                    if c + 1 < na:
                        _proj_qk(
                            nc, wqk_sb, qkpool, pp_proj, nxt_x, 2, 4,
                            qkT=qkT_n,
                        )
                    _group(nc, 2, ebias_sb, vts, qkT, ot,
                           epool, rpool, pp_s, pp_o, EXP)
                    if c + 1 < na:
                        _proj_qk(
                            nc, wqk_sb, qkpool, pp_proj, nxt_x, 4, 6,
                            qkT=qkT_n,
                        )
                    if c < nv:
                        _v_chunk(
                            nc, c, xvT, out_v, wv_sb, xpool, ovpool, pp_proj
                        )
                    _group(nc, 3, ebias_sb, vts, qkT, ot,
                           epool, rpool, pp_s, pp_o, EXP)
                    if c + 1 < na:
                        _proj_qk(
                            nc, wqk_sb, qkpool, pp_proj, nxt_x, 6, 8,
                            qkT=qkT_n,
                        )
                    # out rows for chunk c: token rows c*512 + g*128 + p2*64
                    # + t, features hb*256 + cc; ot = [part=(hb,t),
                    # col=(g,p2,cc)]
                    row0 = c * 512
                    for hb in range(2):
                        src = ot[hb * 64 : (hb + 1) * 64, :].rearrange(
                            "t (g p2 cc) -> t g p2 cc", g=4, p2=2
                        )
                        dst = out_a[
                            row0 : row0 + 512, hb * 256 : (hb + 1) * 256
                        ].rearrange("(g p2 t) cc -> t g p2 cc", g=4, p2=2)
                        nc.sync.dma_start(dst, src)
                    staged = (nxt_v, qkT_n)

    nc.finalize()
    return nc


def _proj_v(nc, c, xaT, wv_sb, xpool, vpool, pp_proj, xT_pre=None):
    if xT_pre is not None:
        xT = xT_pre
    else:
        xaT_r = xaT[:, :].rearrange("(k p) t -> p k t", p=P)
        xT = xpool.tile([P, 4, 512], BF16, tag="xT")
        nc.sync.dma_start(xT[:], xaT_r[:, :, c * 512 : (c + 1) * 512])

    # --- V projection first: its lhsT=xT ldweights absorbs the DMA wait
    vts = []
    for tt in range(4):
        psv = pp_proj.tile([P, 512], F32, tag="ps_proj")
        for kt in range(4):
            nc.tensor.matmul(
                psv[:],
                lhsT=xT[:, kt, tt * 128 : (tt + 1) * 128],
                rhs=wv_sb[:, kt, :],
                start=(kt == 0),
                stop=(kt == 3),
            )
        # persistent ring slot: the ones-column written on first use survives
        vt = vpool.tile([P, 8, 65], BF16, tag=f"vt{(c % 2) * 4 + tt}")
        if c < 2:
            nc.gpsimd.memset(vt[:, :, 64:65], 1.0)
        # alternate the PSUM-drain engine so neither queue backs up
        if tt % 2 == 0:
            nc.vector.tensor_copy(
                out=vt[:, :, 0:64],
                in_=psv[:].rearrange("p (h d) -> p h d", h=8),
            )
        else:
            nc.scalar.copy(
                out=vt[:, :, 0:64],
                in_=psv[:].rearrange("p (h d) -> p h d", h=8),
            )
        vts.append(vt)
    return vts, xT


def _proj_qk(nc, wqk_sb, qkpool, pp_proj, xT, ft_lo, ft_hi, qkT=None):
    # q^T, k^T projection: feature-major (2 heads per 128 partitions)
    if qkT is None:
        qkT = qkpool.tile([P, 8, 512], BF16, tag="qkT")
    for ft in range(ft_lo, ft_hi):
        ps = pp_proj.tile([P, 512], F32, tag="ps_proj")
        for kt in range(4):
            nc.tensor.matmul(
                ps[:],
                lhsT=wqk_sb[:, kt, ft * 128 : (ft + 1) * 128],
                rhs=xT[:, kt, :],
                start=(kt == 0),
                stop=(kt == 3),
            )
        nc.scalar.copy(out=qkT[:, ft, :], in_=ps[:])
    return qkT


def _group(nc, g, ebias_sb, vts, qkT, ot, epool, rpool, pp_s, pp_o, EXP):
    # --- attention, one group of 2 positions (16 (pos,head) pairs)
    # Concurrent matmuls on different PE row-groups must write
    # different PSUM banks (HW hang otherwise):
    #  - sim MMs: row-group = head parity -> 2-bank pss tile, bank by h%2
    #  - PV MMs: row-group = position parity (p2) -> bank by p2; out rows
    #    keyed by head-half (hb) so all 4 quadrants run concurrently.
    pss = pp_s.tile([P, 1024], F32, tag="ps_s")
    for h in range(8):
        ft = h // 2
        pb = (h % 2) * 64
        col0 = (h % 2) * 512 + (h // 2) * 64
        for p2 in range(2):
            tok0 = g * 128 + p2 * 64
            nc.tensor.matmul(
                pss[p2 * 64 : (p2 + 1) * 64, col0 : col0 + 64],
                lhsT=qkT[pb : pb + 64, 4 + ft, tok0 : tok0 + 64],
                rhs=qkT[pb : pb + 64, ft, tok0 : tok0 + 64],
                start=True,
                stop=True,
                tile_position=(pb, p2 * 64),
            )
    # exp(sim + bias) = exp(sim) * exp(bias); bias folded as a
    # multiplicative constant so pss is read by ScalarE only.
    # E col layout: ecol(h) = (h%2)*256 + (h//2)*64
    e_raw = epool.tile([P, 512], BF16, tag="Eraw")
    nc.scalar.activation(
        e_raw[:].rearrange("p (b c) -> p b c", b=2),
        pss[:].rearrange("p (b c) -> p b c", b=2)[:, :, 0:256],
        EXP,
    )
    e_t = epool.tile([P, 512], BF16, tag="E")
    nc.vector.tensor_tensor(
        e_t[:], e_raw[:], ebias_sb[:], mybir.AluOpType.mult
    )

    pvt = pp_o.tile([P, 1024], F32, tag="pvt")
    vt = vts[g]
    for h in range(8):
        hb, hh = h // 4, h % 4
        for p2 in range(2):
            nc.tensor.matmul(
                pvt[hb * 64 : (hb + 1) * 64,
                    p2 * 512 + hh * 65 : p2 * 512 + hh * 65 + 65],
                lhsT=e_t[p2 * 64 : (p2 + 1) * 64, _ecol(h) : _ecol(h) + 64],
                rhs=vt[p2 * 64 : (p2 + 1) * 64, h, :],
                start=True,
                stop=True,
                tile_position=(p2 * 64, hb * 64),
            )

    # normalize: denominators sit at col 64 of each 65-block
    pvt_r = (
        pvt[:]
        .rearrange("p (p2 c) -> p p2 c", p2=2)[:, :, 0:260]
        .rearrange("p p2 (h x) -> p p2 h x", h=4)
    )
    rec = rpool.tile([P, 2, 4, 1], F32, tag="rec")
    nc.vector.reciprocal(rec[:], pvt_r[:, :, :, 64:65])
    nc.vector.tensor_tensor(
        ot[:, g * 512 : (g + 1) * 512].rearrange(
            "p (p2 h d) -> p p2 h d", p2=2, h=4
        ),
        pvt_r[:, :, :, 0:64],
        rec[:].to_broadcast((P, 2, 4, 64)),
        mybir.AluOpType.mult,
    )


def _v_chunk(nc, c, xvT, out_v, wv_sb, xpool, ovpool, pp_proj):
    xvT_r = xvT[:, :].rearrange("(k p) t -> p k t", p=P)
    xT = xpool.tile([P, 4, 512], BF16, tag="xT2")
    nc.sync.dma_start(xT[:], xvT_r[:, :, c * 512 : (c + 1) * 512])
    ov = ovpool.tile([P, 4, 512], BF16, tag="ov")
    for tt in range(4):
        psv = pp_proj.tile([P, 512], F32, tag="ps_proj")
        for kt in range(4):
            nc.tensor.matmul(
                psv[:],
                lhsT=xT[:, kt, tt * 128 : (tt + 1) * 128],
                rhs=wv_sb[:, kt, :],
                start=(kt == 0),
                stop=(kt == 3),
            )
        if tt % 2 == 0:
            nc.vector.tensor_copy(out=ov[:, tt, :], in_=psv[:])
        else:
            nc.scalar.copy(out=ov[:, tt, :], in_=psv[:])
        # store each 128-row block as soon as its copy lands
        nc.sync.dma_start(
            out_v[c * 512 + tt * 128 : c * 512 + (tt + 1) * 128, :],
            ov[:, tt, :],
        )


def _pad_positions(idx: np.ndarray, mult: int) -> np.ndarray:
    """Pad a position-index list to a multiple of `mult` by repeating the last
    entry (duplicates are recomputed and harmlessly overwritten on scatter)."""
    if len(idx) % mult == 0:
        return idx
    pad = mult - len(idx) % mult
    return np.concatenate([idx, np.full(pad, idx[-1], dtype=idx.dtype)])


def host_consts(w_qkv, pos_bias):
    """Host-side constant prep shared by kernel() and tests."""
    import ml_dtypes
    bf16 = ml_dtypes.bfloat16
    scale = DH ** -0.5
    wq = w_qkv[0:512] * scale
    wk = w_qkv[512:1024]
    wv = w_qkv[1024:1536]
    wqkT = np.ascontiguousarray(np.concatenate([wq, wk], axis=0).T.astype(bf16))
    wvT = np.ascontiguousarray(wv.T.astype(bf16))
    # ebiasT[p2*64+j, ecol(h)+i] = exp(pos_bias[h, i, j]), ecol = (h%2)*256+(h//2)*64
    big = np.zeros((64, 512), np.float32)
    for h in range(HEADS):
        big[:, _ecol(h) : _ecol(h) + 64] = pos_bias[h].T
    ebiasT = np.ascontiguousarray(np.exp(np.tile(big, (2, 1))).astype(bf16))
    return wqkT, wvT, ebiasT


def kernel(x, w_qkv, pos_bias, focus_present_mask):
    global LAST_RESULT
    _ensure_ntff_hook()
    x = np.ascontiguousarray(np.asarray(x), dtype=np.float32)
    w_qkv = np.asarray(w_qkv, dtype=np.float32)
    pos_bias = np.asarray(pos_bias, dtype=np.float32)
    mask = np.asarray(focus_present_mask).astype(bool)

    b, hw, n, dim = x.shape
    assert (n, dim) == (NTOK, DIM) and w_qkv.shape == (3 * HEADS * DH, DIM)
    x_flat = x.reshape(b * hw, n, dim)

    flat_idx = np.arange(b * hw)
    batch_of = flat_idx // hw
    attn_idx = flat_idx[~mask[batch_of]]
    vpr_idx = flat_idx[mask[batch_of]]

    # per-core granularity: 8 positions (one 512-token chunk) x 8 cores
    attn_idx = _pad_positions(attn_idx, 8 * N_CORES) if len(attn_idx) else attn_idx
    vpr_idx = _pad_positions(vpr_idx, 8 * N_CORES) if len(vpr_idx) else vpr_idx
    a_pos_pc = len(attn_idx) // N_CORES
    v_pos_pc = len(vpr_idx) // N_CORES
    a_tok = a_pos_pc * NTOK
    v_tok = v_pos_pc * NTOK

    key = (a_tok, v_tok)
    if key not in _KERNEL_CACHE:
        _KERNEL_CACHE[key] = _build_kernel(a_tok, v_tok)
    nc = _KERNEL_CACHE[key]

    import ml_dtypes
    bf16 = ml_dtypes.bfloat16
    wqkT, wvT, ebiasT = host_consts(w_qkv, pos_bias)

    in_maps = []
    for core in range(N_CORES):
        m = {"wqkT": wqkT, "wvT": wvT, "ebiasT": ebiasT}
        if a_tok:
            ai = attn_idx[core * a_pos_pc : (core + 1) * a_pos_pc]
            m["xaT"] = np.ascontiguousarray(x_flat[ai].reshape(-1, DIM).T.astype(bf16))
        if v_tok:
            vi = vpr_idx[core * v_pos_pc : (core + 1) * v_pos_pc]
            m["xvT"] = np.ascontiguousarray(x_flat[vi].reshape(-1, DIM).T.astype(bf16))
        in_maps.append(m)

    res = run_bass_kernel_spmd(nc, in_maps, core_ids=list(range(N_CORES)))
    LAST_RESULT = res

    out_flat = np.empty((b * hw, n, HEADS * DH), dtype=np.float32)
    for core in range(N_CORES):
        if a_tok:
            ai = attn_idx[core * a_pos_pc : (core + 1) * a_pos_pc]
            out_flat[ai] = (
                res.results[core]["out_a"]
                .astype(np.float32)
                .reshape(a_pos_pc, n, HEADS * DH)
            )
        if v_tok:
            vi = vpr_idx[core * v_pos_pc : (core + 1) * v_pos_pc]
            out_flat[vi] = (
                res.results[core]["out_v"]
                .astype(np.float32)
                .reshape(v_pos_pc, n, HEADS * DH)
            )
    return out_flat.reshape(b, hw, n, HEADS * DH)

